# revision 15
# baseline (speedup 1.0000x reference)
"""ChebyshevSheafFilter Trainium2 kernel v3 (8 NeuronCores).

Structure (per core, per Chebyshev step):
    gather  : SWDGE dma_gather of x[col[e]] from a sparse bf16 node table in
              HBM (256B slots, 4 int16-indexed classes), one call per
              (group-pair, class) chunked at <=8 blocks, spread across 4
              parallel SWDGE queues (queue = class) for 4x descriptor-gen
              throughput.
    prod    : DVE  prod[e,(j,i)] = Qt[e,(j,i)] * xc[e,j]   (bf16), one instr
              per (pair, class)
    S8      : DVE  one-hot scatter matrix, one instr per pair
    PE      : psum[c,(j,i)] += S8^T @ prod  (segment-sum over edges)
    reduce  : DVE  off[c,i] = sum_j psum[c,(j,i)]
    update  : DVE  Chebyshev recursion on the local node slice (f32)
    AllGather (dense bf16, 3.2MB) -> local expand -> sparse table rebuild

Node mapping: core c owns nodes [c*12500,(c+1)*12500); local node r sits at
SBUF (partition, slot) = (r//98, r%98); dest group = r%98 (128 nodes across
partitions) so segment-sums land PSUM-partition-aligned with the node state.
Blocks are ordered (pair, class, group, block) so gather/prod/S8 slices are
contiguous per pair.
"""

import math
import os

import numpy as np

import concourse.bacc as bacc
import concourse.bass as bass
import concourse.mybir as mybir
import concourse.tile as tile
from concourse.bass_utils import run_bass_kernel_spmd

N_CORES = 8
N = 100000
E_TOT = 1000000
D = 16
DD = 256
ORDER = 4
P = 128
NPC = N // N_CORES          # 12500
NG = 98                     # dest groups per core (= slots per partition)
NPAIR = NG // 2             # 49 psum pairs
SLOTS = NG * P              # 12544 padded local slots
SUBT = 25088                # class size (4*25088=100352, max idx < int16 max)
N_CLS = 4
MAX_GBLK = 8                # max blocks per dma_gather call (1024 idxs)
N_QUEUES = 4

ABLATE = set()

# ---------------------------------------------------------------------------
# walrus workaround: builds reject instructions with >1 semaphore wait.
_spill_counter = [0]


def _spill_excess_waits(nc, max_waits=1):
    n_spilled = 0
    for fn in nc.m.functions:
        for bb in fn.blocks:
            insts = list(bb.instructions)
            new_list = []
            changed = False
            for inst in insts:
                si = inst.sync_info
                waits = list(si.on_wait) if si is not None and si.on_wait else []
                if len(waits) > max_waits:
                    changed = True
                    keep = waits[-max_waits:]
                    overflow = waits[: len(waits) - max_waits]
                    for i in range(0, len(overflow), max_waits):
                        chunk = overflow[i : i + max_waits]
                        _spill_counter[0] += 1
                        nop = mybir.InstNoOp(
                            name=f"waitspill-{_spill_counter[0]}",
                            engine=inst.engine,
                            bass_nofuse=True,
                            sync_info=mybir.SyncInfo(on_wait=chunk, on_update=[]),
                        )
                        new_list.append(nop)
                        n_spilled += 1
                    si.on_wait = keep
                new_list.append(inst)
            if changed:
                live = bb.instructions
                live.clear()
                live.extend(new_list)
    return n_spilled


# ---------------------------------------------------------------------------
# host preprocessing


def _wrap_idx16(flat):
    n = flat.shape[0]
    assert n % 16 == 0
    w = np.zeros((16, n // 16), dtype=np.int16)
    for p in range(16):
        w[p] = flat[p::16]
    return np.tile(w, (8, 1))


def _preprocess(h, Q, coeffs, edge_index, lambda_max):
    import ml_dtypes

    bf16 = ml_dtypes.bfloat16

    row = np.asarray(edge_index[0], dtype=np.int64)
    col = np.asarray(edge_index[1], dtype=np.int64)
    h = np.asarray(h, dtype=np.float32)
    Q = np.asarray(Q, dtype=np.float32)

    lam = float(np.asarray(lambda_max, dtype=np.float64))
    alpha = 2.0 / (lam + 1e-8)
    cf = np.asarray(coeffs, dtype=np.float64)
    w = np.exp(cf - cf.max())
    w = w / w.sum()

    degree = (
        np.bincount(row, minlength=N) + np.bincount(col, minlength=N)
    ).astype(np.float32)

    c_e = row // NPC
    r_e = row % NPC
    g_e = r_e % NG
    rowl_e = (r_e // NG).astype(np.float32)

    slot_col = (col // NPC) * SLOTS + (col % NPC)
    cls_e = slot_col // SUBT
    idx16_e = (slot_col % SUBT).astype(np.int16)

    # bucket (core, group, class); rank within bucket
    key = (c_e * NG + g_e) * N_CLS + cls_e
    perm = np.argsort(key, kind="stable")
    key_s = key[perm]

    n_seg = N_CORES * NG * N_CLS
    sizes = np.bincount(key_s, minlength=n_seg).reshape(N_CORES, NG * N_CLS)
    NB_gt = ((sizes.max(axis=0) + P - 1) // P).astype(np.int64).reshape(NG, N_CLS)
    empty_g = NB_gt.sum(axis=1) == 0
    NB_gt[empty_g, 0] = 1

    # global block order: (pair, class, group-in-pair, block)
    base_gt = np.zeros((NG, N_CLS), dtype=np.int64)
    nb_pt = np.zeros((NPAIR, N_CLS), dtype=np.int64)
    blk = 0
    for p2 in range(NPAIR):
        for t in range(N_CLS):
            for g in (2 * p2, 2 * p2 + 1):
                base_gt[g, t] = blk
                blk += int(NB_gt[g, t])
            nb_pt[p2, t] = NB_gt[2 * p2, t] + NB_gt[2 * p2 + 1, t]
    nblk = blk
    nb_pair = nb_pt.sum(axis=1)
    # block base of each (pair, class) and pair
    base_pt = np.zeros((NPAIR, N_CLS), dtype=np.int64)
    base_p = np.zeros(NPAIR + 1, dtype=np.int64)
    for p2 in range(NPAIR):
        base_p[p2] = base_gt[2 * p2, 0]
        for t in range(N_CLS):
            base_pt[p2, t] = base_gt[2 * p2, t]
    base_p[NPAIR] = nblk

    seg_start = np.zeros(n_seg, dtype=np.int64)
    np.cumsum(sizes.reshape(-1)[:-1], out=seg_start[1:])
    rank = np.arange(E_TOT, dtype=np.int64) - seg_start[key_s]
    p_e = rank % P
    gt_of_edge = key_s % (NG * N_CLS)
    g_of_edge = gt_of_edge // N_CLS
    t_of_edge = gt_of_edge % N_CLS
    blk_e = base_gt.reshape(-1)[gt_of_edge] + rank // P

    # Q transposed per edge: flat (j, i) layout so prod broadcasts innermost
    QF = np.ascontiguousarray(Q.transpose(0, 2, 1)).reshape(E_TOT, DD)

    in_maps = []
    for c in range(N_CORES):
        m = (key_s // (NG * N_CLS)) == c
        e_idx = perm[m]
        p_c = p_e[m]
        blk_c = blk_e[m]

        qs2 = np.zeros((P, nblk, DD), dtype=bf16)
        qs2[p_c, blk_c] = QF[e_idx].astype(bf16)

        rowl_t = np.zeros((P, nblk), dtype=bf16)
        rowl_t[p_c, blk_c] = rowl_e[e_idx].astype(bf16)

        # host-side gather of step 1's stage data (x = h is known)
        st1 = np.zeros((P, nblk, D), dtype=bf16)
        st1[p_c, blk_c] = h[col[e_idx]].astype(bf16)

        stream = np.zeros(nblk * P, dtype=np.int16)
        stream[blk_c * P + p_c] = idx16_e[e_idx]
        idx_arr = _wrap_idx16(stream)
        assert idx_arr.shape == (P, nblk * 8)

        hc = np.zeros((SLOTS, D), dtype=np.float32)
        hc[:NPC] = h[c * NPC : (c + 1) * NPC]
        x0 = np.ascontiguousarray(hc.reshape(P, NG * D))

        dg = np.zeros(SLOTS, dtype=np.float32)
        dg[:NPC] = degree[c * NPC : (c + 1) * NPC]
        deg_t = np.ascontiguousarray(dg.reshape(P, NG))

        in_maps.append(
            {
                "qs2": qs2.reshape(P, nblk * DD),
                "rowl_t": rowl_t,
                "idx_in": idx_arr,
                "deg_t": deg_t,
                "x0": x0,
                "st1": np.ascontiguousarray(st1.reshape(P, nblk * D)),
            }
        )

    # dense h packed partition-major: hd_p[p, c*NG*D + s*D + i] = h at (c, p, s)
    hd_p = np.zeros((N_CORES, P, NG, D), dtype=bf16)
    for c in range(N_CORES):
        hc2 = np.zeros((SLOTS, D), dtype=np.float32)
        hc2[:NPC] = h[c * NPC : (c + 1) * NPC]
        hd_p[c] = hc2.reshape(P, NG, D).astype(bf16)
    hd_p = np.ascontiguousarray(hd_p.transpose(1, 0, 2, 3)).reshape(
        P, N_CORES * NG * D
    )
    iota = np.tile(np.arange(P, dtype=bf16), (P, 1))
    for c in range(N_CORES):
        im = in_maps[c]
        fblob = np.concatenate([im.pop("deg_t"), im.pop("x0")], axis=1)
        pad1 = np.zeros((P, 1), dtype=bf16)
        im["bq"] = np.concatenate(
            [im.pop("qs2"), hd_p, im.pop("idx_in").view(bf16),
             im.pop("st1"), im.pop("rowl_t"), pad1, iota, fblob.view(bf16)],
            axis=1,
        )

    meta = dict(
        nblk=nblk,
        NB_gt=NB_gt,
        base_gt=base_gt,
        nb_pt=nb_pt,
        base_pt=base_pt,
        base_p=base_p,
        nb_pair=nb_pair,
        alpha=alpha,
        w=[float(x) for x in w],
    )
    return in_maps, meta


# ---------------------------------------------------------------------------
# device program


def _build_nc(meta):
    nblk = meta["nblk"]
    NB_gt = meta["NB_gt"]
    base_gt = meta["base_gt"]
    nb_pt = meta["nb_pt"]
    base_pt = meta["base_pt"]
    base_p = meta["base_p"]
    nb_pair = meta["nb_pair"]
    alpha = meta["alpha"]
    w = meta["w"]
    GF = NG * D  # 1568
    TROWS = N_CORES * SLOTS  # 100352
    max_nb_pt = int(nb_pt.max())
    max_nb_pair = int(nb_pair.max())

    f32 = mybir.dt.float32
    bf16 = mybir.dt.bfloat16
    i16 = mybir.dt.int16
    nc = bacc.Bacc(None, num_swdge_queues=N_QUEUES)

    FB = NG + GF  # f32 blob columns
    bq = nc.declare_dram_parameter(
        "bq",
        [P, nblk * DD + N_CORES * NG * D + nblk * 8 + nblk * D
         + nblk + 1 + P + 2 * FB],
        bf16,
        isOutput=False,
    )
    r_out = nc.declare_dram_parameter("r_out", [P, GF], f32, isOutput=True)
    qs2 = bq[:, : nblk * DD]
    hd_in = bq[:, nblk * DD : nblk * DD + N_CORES * NG * D]
    _ioff = nblk * DD + N_CORES * NG * D
    bi = bq[:, _ioff : _ioff + nblk * 8].bitcast(i16)
    _o2 = _ioff + nblk * 8
    st1_in = bq[:, _o2 : _o2 + nblk * D]
    rowl_in = bq[:, _o2 + nblk * D : _o2 + nblk * D + nblk]
    iota_in = bq[:, _o2 + nblk * D + nblk + 1 : _o2 + nblk * D + nblk + 1 + P]
    bf = bq[:, _o2 + nblk * D + nblk + 1 + P :].bitcast(f32)
    deg_in = bf[:, :NG]
    x0_in = bf[:, NG : NG + GF]
    idx_in = bi

    table = nc.dram_tensor("table", [TROWS, P], bf16)
    ag_in = [nc.dram_tensor(f"ag_in{k}", [SLOTS, D], bf16) for k in (1, 2, 3)]
    ag_out = [
        nc.dram_tensor(f"ag_out{k}", [TROWS, D], bf16, addr_space="Shared")
        for k in (1, 2, 3)
    ]

    with tile.TileContext(nc) as tc:
        with (
            tc.tile_pool(name="const", bufs=1) as const_pool,
            tc.tile_pool(name="state", bufs=1) as state_pool,
            tc.tile_pool(name="dense", bufs=2) as dense_pool,
            tc.tile_pool(name="expand", bufs=2) as exp_pool,
            tc.tile_pool(name="qg", bufs=3) as q_pool,
            tc.tile_pool(name="stage", bufs=2) as stage_pool,
            tc.tile_pool(name="prod", bufs=2) as prod_pool,
            tc.tile_pool(name="sgen", bufs=2) as s_pool,
            tc.tile_pool(name="off", bufs=2) as off_pool,
            tc.tile_pool(name="upd", bufs=2) as upd_pool,
            tc.tile_pool(name="cast", bufs=2) as cast_pool,
            tc.tile_pool(name="psum", bufs=6, space="PSUM") as psum_pool,
        ):
            rowl_t = const_pool.tile([P, nblk], bf16)
            deg_t = const_pool.tile([P, NG], f32)
            iota_t = const_pool.tile([P, P], bf16)
            idx_t = const_pool.tile([P, nblk * 8], i16)
            nc.sync.dma_start(out=idx_t[:], in_=idx_in[:])
            nc.sync.dma_start(out=rowl_t[:], in_=rowl_in[:])
            nc.sync.dma_start(out=deg_t[:], in_=deg_in[:])
            nc.sync.dma_start(out=iota_t[:], in_=iota_in[:])

            xa = state_pool.tile([P, GF], f32, tag="xa")
            xb = state_pool.tile([P, GF], f32, tag="xb")
            r_t = state_pool.tile([P, GF], f32, tag="r")
            nc.sync.dma_start(out=xa[:], in_=x0_in[:])
            x, tp = xa, xb

            nreg = {}

            def get_nreg(n):
                if n not in nreg:
                    nreg[n] = nc.gpsimd.snap(n)
                return nreg[n]

            # prime registers for every chunk size used
            for p2 in range(NPAIR):
                for t in range(N_CLS):
                    nb = int(nb_pt[p2, t])
                    for b0 in range(0, nb, MAX_GBLK):
                        get_nreg(min(MAX_GBLK, nb - b0) * P)

            def build_table(src):
                # src: dense bf16; param layout [P, c*(NG*D)] or ag [TROWS, D]
                for c in range(N_CORES):
                    dt_ = dense_pool.tile([P, NG * D], bf16, tag="dense")
                    if src is None:
                        nc.scalar.dma_start(
                            out=dt_[:],
                            in_=hd_in[:, c * NG * D : (c + 1) * NG * D],
                        )
                    else:
                        nc.scalar.dma_start(
                            out=dt_[:],
                            in_=src[c * SLOTS : (c + 1) * SLOTS, :].rearrange(
                                "(p s) i -> p (s i)", p=P
                            ),
                        )
                    for hh in range(2):
                        s0, s1 = hh * (NG // 2), (hh + 1) * (NG // 2) if hh == 0 else NG
                        ns = s1 - s0
                        et = exp_pool.tile([P, (NG - NG // 2) * P], bf16, tag="exp")
                        nc.vector.tensor_copy(
                            out=et[:, : ns * P].rearrange(
                                "p (s e) -> p s e", e=P
                            )[:, :, :D],
                            in_=dt_[:, s0 * D : s1 * D].rearrange(
                                "p (s i) -> p s i", i=D
                            ),
                        )
                        nc.scalar.dma_start(
                            out=table[c * SLOTS : (c + 1) * SLOTS, :]
                            .rearrange("(p s) e -> p s e", p=P)[:, s0:s1, :],
                            in_=et[:, : ns * P].rearrange(
                                "p (s e) -> p s e", e=P
                            ),
                        )

            for k in range(1, ORDER + 1):
                wk = w[k]
                off_full = off_pool.tile([P, GF], f32, tag="off")
                for p2 in range(NPAIR):
                    nbp = int(nb_pair[p2])
                    bp = int(base_p[p2])

                    # Q for the whole pair, (t, g, b)-ordered
                    qg = q_pool.tile([P, max_nb_pair * DD], bf16, tag="qg")
                    if "qdma" not in ABLATE:
                        nc.sync.dma_start(
                            out=qg[:, : nbp * DD],
                            in_=qs2[:, bp * DD : (bp + nbp) * DD],
                        )

                    # one-hot scatter matrix for the whole pair
                    S8 = s_pool.tile([P, max_nb_pair * P], bf16, tag="S8")
                    if "s8" not in ABLATE:
                        nc.vector.tensor_tensor(
                            out=S8[:, : nbp * P].rearrange(
                                "p (b c) -> p b c", b=nbp
                            ),
                            in0=rowl_t[:, bp : bp + nbp]
                            .unsqueeze(2)
                            .to_broadcast([P, nbp, P]),
                            in1=iota_t[:].unsqueeze(1).to_broadcast([P, nbp, P]),
                            op=mybir.AluOpType.is_equal,
                        )

                    pacc = psum_pool.tile([P, 2 * DD], f32, tag="acc")

                    stg = {}
                    for t in range(N_CLS):
                        nb = int(nb_pt[p2, t])
                        if nb == 0:
                            continue
                        bpt = int(base_pt[p2, t])
                        if k == 1:
                            # step 1 gathers x = h: pre-gathered on host
                            st = stage_pool.tile(
                                [P, max_nb_pt * D], bf16, tag=f"s1{t}"
                            )
                            nc.sync.dma_start(
                                out=st[:, : nb * D],
                                in_=st1_in[:, bpt * D : (bpt + nb) * D],
                            )
                            in1 = (
                                st[:, : nb * D]
                                .rearrange("p (b j) -> p b j", j=D)
                                .unsqueeze(3)
                                .to_broadcast([P, nb, D, D])
                            )
                        else:
                            st = stage_pool.tile(
                                [P, max_nb_pt * P], bf16, tag=f"st{t}"
                            )
                            if "gather" not in ABLATE:
                                for b0 in range(0, nb, MAX_GBLK):
                                    bn = min(MAX_GBLK, nb - b0)
                                    nc.gpsimd.dma_gather(
                                        out_ap=st[
                                            :, b0 * P : (b0 + bn) * P
                                        ].rearrange("p (b e) -> p b e", e=P),
                                        in_ap=table[
                                            t * SUBT : (t + 1) * SUBT, :
                                        ],
                                        idxs_ap=idx_t[
                                            :,
                                            (bpt + b0) * 8 : (bpt + b0 + bn) * 8,
                                        ],
                                        num_idxs=bn * P,
                                        num_idxs_reg=get_nreg(bn * P),
                                        elem_size=P,
                                        queue_num=t % N_QUEUES,
                                    )
                            in1 = (
                                st[:, : nb * P]
                                .rearrange("p (b e) -> p b e", e=P)[:, :, :D]
                                .unsqueeze(3)
                                .to_broadcast([P, nb, D, D])
                            )

                        prod = prod_pool.tile(
                            [P, max_nb_pt * DD], bf16, tag=f"prod{t}"
                        )
                        if "prod" not in ABLATE:
                            nc.vector.tensor_tensor(
                                out=prod[:, : nb * DD].rearrange(
                                    "p (b j i) -> p b j i", b=nb, j=D
                                ),
                                in0=qg[
                                    :, (bpt - bp) * DD : (bpt - bp + nb) * DD
                                ].rearrange("p (b j i) -> p b j i", b=nb, j=D),
                                in1=in1,
                                op=mybir.AluOpType.mult,
                            )
                        stg[t] = (st, prod)

                    if "mm" not in ABLATE:
                        # per-g start/stop across the (t, g, b) block order
                        for gi, g in enumerate((2 * p2, 2 * p2 + 1)):
                            acc = pacc[:, gi * DD : (gi + 1) * DD]
                            g_blocks = []
                            for t in range(N_CLS):
                                nbg = int(NB_gt[g, t])
                                if nbg == 0 or t not in stg:
                                    continue
                                bgt = int(base_gt[g, t])
                                bpt = int(base_pt[p2, t])
                                for b in range(nbg):
                                    g_blocks.append((t, bgt - bpt + b, bgt + b))
                            for bi_, (t, loc_t, glob) in enumerate(g_blocks):
                                nc.tensor.matmul(
                                    out=acc,
                                    lhsT=S8[
                                        :, (glob - bp) * P : (glob - bp + 1) * P
                                    ],
                                    rhs=stg[t][1][
                                        :, loc_t * DD : (loc_t + 1) * DD
                                    ],
                                    start=(bi_ == 0),
                                    stop=(bi_ == len(g_blocks) - 1),
                                )
                        nc.vector.tensor_reduce(
                            out=off_full[
                                :, (2 * p2) * D : (2 * p2 + 2) * D
                            ].rearrange("p (u i) -> p u i", u=2),
                            in_=pacc[:].rearrange(
                                "p (u j i) -> p u i j", u=2, j=D
                            ),
                            axis=mybir.AxisListType.X,
                            op=mybir.AluOpType.add,
                        )

                # ---- node update ----
                tmp = upd_pool.tile([P, GF], f32, tag="tmp")
                nc.vector.tensor_tensor(
                    out=tmp[:].rearrange("p (s i) -> p s i", s=NG),
                    in0=x[:].rearrange("p (s i) -> p s i", s=NG),
                    in1=deg_t[:].unsqueeze(2).to_broadcast([P, NG, D]),
                    op=mybir.AluOpType.mult,
                )
                nc.vector.tensor_tensor(
                    out=tmp[:], in0=tmp[:], in1=off_full[:],
                    op=mybir.AluOpType.subtract,
                )
                nc.vector.tensor_scalar_mul(tmp[:], tmp[:], alpha)
                tmp2 = upd_pool.tile([P, GF], f32, tag="tmp2")
                if k == 1:
                    nc.vector.tensor_tensor(
                        out=tp[:], in0=tmp[:], in1=x[:], op=mybir.AluOpType.subtract
                    )
                    nc.vector.tensor_scalar_mul(r_t[:], x[:], w[0])
                    nc.vector.tensor_scalar_mul(tmp2[:], tp[:], wk)
                    nc.vector.tensor_tensor(
                        out=r_t[:], in0=r_t[:], in1=tmp2[:], op=mybir.AluOpType.add
                    )
                else:
                    nc.vector.tensor_tensor(
                        out=tmp[:], in0=tmp[:], in1=x[:], op=mybir.AluOpType.subtract
                    )
                    nc.vector.tensor_scalar_mul(tmp[:], tmp[:], 2.0)
                    nc.vector.tensor_tensor(
                        out=tp[:], in0=tmp[:], in1=tp[:], op=mybir.AluOpType.subtract
                    )
                    nc.vector.tensor_scalar_mul(tmp2[:], tp[:], wk)
                    nc.vector.tensor_tensor(
                        out=r_t[:], in0=r_t[:], in1=tmp2[:], op=mybir.AluOpType.add
                    )
                x, tp = tp, x

                if k < ORDER:
                    ct = cast_pool.tile([P, GF], bf16, tag="ct")
                    nc.vector.tensor_copy(out=ct[:], in_=x[:])
                    dst = ag_in[k - 1]
                    nc.scalar.dma_start(
                        out=dst[:].rearrange("(p s) i -> p (s i)", p=P),
                        in_=ct[:],
                    )
                    if "cc" not in ABLATE:
                        nc.gpsimd.collective_compute(
                            "AllGather",
                            mybir.AluOpType.bypass,
                            ins=[dst[:]],
                            outs=[ag_out[k - 1][:]],
                            replica_groups=[list(range(N_CORES))],
                        )
                    build_table(ag_out[k - 1])

            nc.sync.dma_start(out=r_out[:], in_=r_t[:])

    nc.compile()
    return nc


# ---------------------------------------------------------------------------

_CACHE = {}


def get_nc(meta):
    key = ("nc", meta["nblk"])
    if key not in _CACHE:
        nc = _build_nc(meta)
        _spill_excess_waits(nc)
        _CACHE[key] = nc
    return _CACHE[key]


def kernel(h, Q, coeffs, edge_index, lambda_max):
    import time as _time

    h = np.asarray(h)
    Q = np.asarray(Q)
    coeffs = np.asarray(coeffs)
    edge_index = np.asarray(edge_index)
    lambda_max = np.asarray(lambda_max)

    _t0 = _time.time()
    in_maps, meta = _preprocess(h, Q, coeffs, edge_index, lambda_max)
    _t1 = _time.time()
    nc = get_nc(meta)
    _t2 = _time.time()
    print(
        f"[kernel] preprocess {_t1-_t0:.1f}s  build+compile {_t2-_t1:.1f}s  "
        f"nblk={meta['nblk']}",
        flush=True,
    )

    res = run_bass_kernel_spmd(nc, in_maps, list(range(N_CORES)))
    print(f"[kernel] run {_time.time()-_t2:.1f}s", flush=True)

    out = np.empty((N, D), dtype=np.float32)
    for c in range(N_CORES):
        r = res.results[c]["r_out"]
        r3 = r.reshape(SLOTS, D)
        out[c * NPC : (c + 1) * NPC] = r3[:NPC]
    return out


# revision 18
# speedup vs baseline: 1.3340x; 1.3340x over previous
"""ChebyshevSheafFilter Trainium2 kernel v3 (8 NeuronCores).

Structure (per core, per Chebyshev step):
    gather  : SWDGE dma_gather of x[col[e]] from a sparse bf16 node table in
              HBM (256B slots, 4 int16-indexed classes), one call per
              (group-pair, class) chunked at <=8 blocks, spread across 4
              parallel SWDGE queues (queue = class) for 4x descriptor-gen
              throughput.
    prod    : DVE  prod[e,(j,i)] = Qt[e,(j,i)] * xc[e,j]   (bf16), one instr
              per (pair, class)
    S8      : DVE  one-hot scatter matrix, one instr per pair
    PE      : psum[c,(j,i)] += S8^T @ prod  (segment-sum over edges)
    reduce  : DVE  off[c,i] = sum_j psum[c,(j,i)]
    update  : DVE  Chebyshev recursion on the local node slice (f32)
    AllGather (dense bf16, 3.2MB) -> local expand -> sparse table rebuild

Node mapping: core c owns nodes [c*12500,(c+1)*12500); local node r sits at
SBUF (partition, slot) = (r//98, r%98); dest group = r%98 (128 nodes across
partitions) so segment-sums land PSUM-partition-aligned with the node state.
Blocks are ordered (pair, class, group, block) so gather/prod/S8 slices are
contiguous per pair.
"""

import math
import os

import numpy as np

import concourse.bacc as bacc
import concourse.bass as bass
import concourse.mybir as mybir
import concourse.tile as tile
from concourse.bass_utils import run_bass_kernel_spmd

N_CORES = 8
N = 100000
E_TOT = 1000000
D = 16
DD = 256
ORDER = 4
P = 128
NPC = N // N_CORES          # 12500
NG = 98                     # dest groups per core (= slots per partition)
NPAIR = NG // 2             # 49 psum pairs
SLOTS = NG * P              # 12544 padded local slots
SUBT = 25088                # class size (4*25088=100352, max idx < int16 max)
N_CLS = 4
MAX_GBLK = 8                # max blocks per dma_gather call (1024 idxs)
N_QUEUES = int(os.environ.get("K_NQUEUES", "4"))
USE_ST1 = os.environ.get("K_ST1", "1") == "1"

ABLATE = set()

# ---------------------------------------------------------------------------
# walrus workaround: builds reject instructions with >1 semaphore wait.
_spill_counter = [0]


def _spill_excess_waits(nc, max_waits=1):
    n_spilled = 0
    for fn in nc.m.functions:
        for bb in fn.blocks:
            insts = list(bb.instructions)
            new_list = []
            changed = False
            for inst in insts:
                si = inst.sync_info
                waits = list(si.on_wait) if si is not None and si.on_wait else []
                if len(waits) > max_waits:
                    changed = True
                    keep = waits[-max_waits:]
                    overflow = waits[: len(waits) - max_waits]
                    for i in range(0, len(overflow), max_waits):
                        chunk = overflow[i : i + max_waits]
                        _spill_counter[0] += 1
                        nop = mybir.InstNoOp(
                            name=f"waitspill-{_spill_counter[0]}",
                            engine=inst.engine,
                            bass_nofuse=True,
                            sync_info=mybir.SyncInfo(on_wait=chunk, on_update=[]),
                        )
                        new_list.append(nop)
                        n_spilled += 1
                    si.on_wait = keep
                new_list.append(inst)
            if changed:
                live = bb.instructions
                live.clear()
                live.extend(new_list)
    return n_spilled


# ---------------------------------------------------------------------------
# host preprocessing


def _wrap_idx16(flat):
    n = flat.shape[0]
    assert n % 16 == 0
    w = np.zeros((16, n // 16), dtype=np.int16)
    for p in range(16):
        w[p] = flat[p::16]
    return np.tile(w, (8, 1))


def _preprocess(h, Q, coeffs, edge_index, lambda_max):
    import ml_dtypes

    bf16 = ml_dtypes.bfloat16

    row = np.asarray(edge_index[0], dtype=np.int64)
    col = np.asarray(edge_index[1], dtype=np.int64)
    h = np.asarray(h, dtype=np.float32)
    Q = np.asarray(Q, dtype=np.float32)

    lam = float(np.asarray(lambda_max, dtype=np.float64))
    alpha = 2.0 / (lam + 1e-8)
    cf = np.asarray(coeffs, dtype=np.float64)
    w = np.exp(cf - cf.max())
    w = w / w.sum()

    degree = (
        np.bincount(row, minlength=N) + np.bincount(col, minlength=N)
    ).astype(np.float32)

    c_e = row // NPC
    r_e = row % NPC
    g_e = r_e % NG
    rowl_e = (r_e // NG).astype(np.float32)

    slot_col = (col // NPC) * SLOTS + (col % NPC)
    cls_e = slot_col // SUBT
    idx16_e = (slot_col % SUBT).astype(np.int16)

    # bucket (core, group, class); rank within bucket
    key = (c_e * NG + g_e) * N_CLS + cls_e
    perm = np.argsort(key, kind="stable")
    key_s = key[perm]

    n_seg = N_CORES * NG * N_CLS
    sizes = np.bincount(key_s, minlength=n_seg).reshape(N_CORES, NG * N_CLS)
    NB_gt = ((sizes.max(axis=0) + P - 1) // P).astype(np.int64).reshape(NG, N_CLS)
    empty_g = NB_gt.sum(axis=1) == 0
    NB_gt[empty_g, 0] = 1

    # global block order: (pair, class, group-in-pair, block)
    base_gt = np.zeros((NG, N_CLS), dtype=np.int64)
    nb_pt = np.zeros((NPAIR, N_CLS), dtype=np.int64)
    blk = 0
    for p2 in range(NPAIR):
        for t in range(N_CLS):
            for g in (2 * p2, 2 * p2 + 1):
                base_gt[g, t] = blk
                blk += int(NB_gt[g, t])
            nb_pt[p2, t] = NB_gt[2 * p2, t] + NB_gt[2 * p2 + 1, t]
    nblk = blk
    nb_pair = nb_pt.sum(axis=1)
    # block base of each (pair, class) and pair
    base_pt = np.zeros((NPAIR, N_CLS), dtype=np.int64)
    base_p = np.zeros(NPAIR + 1, dtype=np.int64)
    for p2 in range(NPAIR):
        base_p[p2] = base_gt[2 * p2, 0]
        for t in range(N_CLS):
            base_pt[p2, t] = base_gt[2 * p2, t]
    base_p[NPAIR] = nblk

    seg_start = np.zeros(n_seg, dtype=np.int64)
    np.cumsum(sizes.reshape(-1)[:-1], out=seg_start[1:])
    rank = np.arange(E_TOT, dtype=np.int64) - seg_start[key_s]
    p_e = rank % P
    gt_of_edge = key_s % (NG * N_CLS)
    g_of_edge = gt_of_edge // N_CLS
    t_of_edge = gt_of_edge % N_CLS
    blk_e = base_gt.reshape(-1)[gt_of_edge] + rank // P

    # Q transposed per edge: flat (j, i) layout so prod broadcasts innermost
    QF = np.ascontiguousarray(Q.transpose(0, 2, 1)).reshape(E_TOT, DD)

    in_maps = []
    for c in range(N_CORES):
        m = (key_s // (NG * N_CLS)) == c
        e_idx = perm[m]
        p_c = p_e[m]
        blk_c = blk_e[m]

        qs2 = np.zeros((P, nblk, DD), dtype=bf16)
        qs2[p_c, blk_c] = QF[e_idx].astype(bf16)

        rowl_t = np.zeros((P, nblk), dtype=bf16)
        rowl_t[p_c, blk_c] = rowl_e[e_idx].astype(bf16)

        # host-side gather of step 1's stage data (x = h is known)
        st1 = np.zeros((P, nblk, D), dtype=bf16)
        st1[p_c, blk_c] = h[col[e_idx]].astype(bf16)

        stream = np.zeros(nblk * P, dtype=np.int16)
        stream[blk_c * P + p_c] = idx16_e[e_idx]
        idx_arr = _wrap_idx16(stream)
        assert idx_arr.shape == (P, nblk * 8)

        hc = np.zeros((SLOTS, D), dtype=np.float32)
        hc[:NPC] = h[c * NPC : (c + 1) * NPC]
        x0 = np.ascontiguousarray(hc.reshape(P, NG * D))

        dg = np.zeros(SLOTS, dtype=np.float32)
        dg[:NPC] = degree[c * NPC : (c + 1) * NPC]
        deg_t = np.ascontiguousarray(dg.reshape(P, NG))

        in_maps.append(
            {
                "qs2": qs2.reshape(P, nblk * DD),
                "rowl_t": rowl_t,
                "idx_in": idx_arr,
                "deg_t": deg_t,
                "x0": x0,
                "st1": np.ascontiguousarray(st1.reshape(P, nblk * D)),
            }
        )

    # dense h packed partition-major: hd_p[p, c*NG*D + s*D + i] = h at (c, p, s)
    hd_p = np.zeros((N_CORES, P, NG, D), dtype=bf16)
    for c in range(N_CORES):
        hc2 = np.zeros((SLOTS, D), dtype=np.float32)
        hc2[:NPC] = h[c * NPC : (c + 1) * NPC]
        hd_p[c] = hc2.reshape(P, NG, D).astype(bf16)
    hd_p = np.ascontiguousarray(hd_p.transpose(1, 0, 2, 3)).reshape(
        P, N_CORES * NG * D
    )
    iota = np.tile(np.arange(P, dtype=bf16), (P, 1))
    for c in range(N_CORES):
        im = in_maps[c]
        fblob = np.concatenate([im.pop("deg_t"), im.pop("x0")], axis=1)
        pad1 = np.zeros((P, 1), dtype=bf16)
        im["bq"] = np.concatenate(
            [im.pop("qs2"), hd_p, im.pop("idx_in").view(bf16),
             im.pop("st1"), im.pop("rowl_t"), pad1, iota, fblob.view(bf16)],
            axis=1,
        )

    meta = dict(
        nblk=nblk,
        NB_gt=NB_gt,
        base_gt=base_gt,
        nb_pt=nb_pt,
        base_pt=base_pt,
        base_p=base_p,
        nb_pair=nb_pair,
        alpha=alpha,
        w=[float(x) for x in w],
    )
    return in_maps, meta


# ---------------------------------------------------------------------------
# device program


def _build_nc(meta):
    nblk = meta["nblk"]
    NB_gt = meta["NB_gt"]
    base_gt = meta["base_gt"]
    nb_pt = meta["nb_pt"]
    base_pt = meta["base_pt"]
    base_p = meta["base_p"]
    nb_pair = meta["nb_pair"]
    alpha = meta["alpha"]
    w = meta["w"]
    GF = NG * D  # 1568
    TROWS = N_CORES * SLOTS  # 100352
    max_nb_pt = int(nb_pt.max())
    max_nb_pair = int(nb_pair.max())

    f32 = mybir.dt.float32
    bf16 = mybir.dt.bfloat16
    i16 = mybir.dt.int16
    nc = bacc.Bacc(None, num_swdge_queues=N_QUEUES)

    FB = NG + GF  # f32 blob columns
    bq = nc.declare_dram_parameter(
        "bq",
        [P, nblk * DD + N_CORES * NG * D + nblk * 8 + nblk * D
         + nblk + 1 + P + 2 * FB],
        bf16,
        isOutput=False,
    )
    r_out = nc.declare_dram_parameter("r_out", [P, GF], f32, isOutput=True)
    qs2 = bq[:, : nblk * DD]
    hd_in = bq[:, nblk * DD : nblk * DD + N_CORES * NG * D]
    _ioff = nblk * DD + N_CORES * NG * D
    bi = bq[:, _ioff : _ioff + nblk * 8].bitcast(i16)
    _o2 = _ioff + nblk * 8
    st1_in = bq[:, _o2 : _o2 + nblk * D]
    rowl_in = bq[:, _o2 + nblk * D : _o2 + nblk * D + nblk]
    iota_in = bq[:, _o2 + nblk * D + nblk + 1 : _o2 + nblk * D + nblk + 1 + P]
    bf = bq[:, _o2 + nblk * D + nblk + 1 + P :].bitcast(f32)
    deg_in = bf[:, :NG]
    x0_in = bf[:, NG : NG + GF]
    idx_in = bi

    table = nc.dram_tensor("table", [TROWS, P], bf16)
    ag_in = [nc.dram_tensor(f"ag_in{k}", [SLOTS, P], bf16) for k in (1, 2, 3)]
    ag_out = [
        nc.dram_tensor(f"ag_out{k}", [TROWS, P], bf16, addr_space="Shared")
        for k in (1, 2, 3)
    ]

    with tile.TileContext(nc) as tc:
        with (
            tc.tile_pool(name="const", bufs=1) as const_pool,
            tc.tile_pool(name="state", bufs=1) as state_pool,
            tc.tile_pool(name="dense", bufs=2) as dense_pool,
            tc.tile_pool(name="expand", bufs=2) as exp_pool,
            tc.tile_pool(name="qg", bufs=3) as q_pool,
            tc.tile_pool(name="stage", bufs=2) as stage_pool,
            tc.tile_pool(name="prod", bufs=2) as prod_pool,
            tc.tile_pool(name="sgen", bufs=2) as s_pool,
            tc.tile_pool(name="off", bufs=2) as off_pool,
            tc.tile_pool(name="upd", bufs=1) as upd_pool,
            tc.tile_pool(name="cast", bufs=1) as cast_pool,
            tc.tile_pool(name="psum", bufs=6, space="PSUM") as psum_pool,
        ):
            rowl_b = const_pool.tile([P, nblk], bf16)
            deg_t = const_pool.tile([P, NG], f32)
            iota_b = const_pool.tile([P, P], bf16)
            idx_t = const_pool.tile([P, nblk * 8], i16)
            nc.sync.dma_start(out=idx_t[:], in_=idx_in[:])
            nc.sync.dma_start(out=rowl_b[:], in_=rowl_in[:])
            nc.sync.dma_start(out=deg_t[:], in_=deg_in[:])
            nc.sync.dma_start(out=iota_b[:], in_=iota_in[:])
            rowl_t = const_pool.tile([P, nblk], f32)
            iota_t = const_pool.tile([P, P], f32)
            nc.vector.tensor_copy(out=rowl_t[:], in_=rowl_b[:])
            nc.vector.tensor_copy(out=iota_t[:], in_=iota_b[:])

            xa = state_pool.tile([P, GF], f32, tag="xa")
            xb = state_pool.tile([P, GF], f32, tag="xb")
            r_t = state_pool.tile([P, GF], f32, tag="r")
            nc.sync.dma_start(out=xa[:], in_=x0_in[:])
            x, tp = xa, xb

            nreg = {}

            def get_nreg(n):
                if n not in nreg:
                    nreg[n] = nc.gpsimd.snap(n)
                return nreg[n]

            # prime registers for every chunk size used
            for p2 in range(NPAIR):
                for t in range(N_CLS):
                    nb = int(nb_pt[p2, t])
                    for b0 in range(0, nb, MAX_GBLK):
                        get_nreg(min(MAX_GBLK, nb - b0) * P)

            def build_table(src):
                # src: dense bf16; param layout [P, c*(NG*D)] or ag [TROWS, D]
                for c in range(N_CORES):
                    dt_ = dense_pool.tile([P, NG * D], bf16, tag="dense")
                    if src is None:
                        nc.scalar.dma_start(
                            out=dt_[:],
                            in_=hd_in[:, c * NG * D : (c + 1) * NG * D],
                        )
                    else:
                        nc.scalar.dma_start(
                            out=dt_[:],
                            in_=src[c * SLOTS : (c + 1) * SLOTS, :].rearrange(
                                "(p s) i -> p (s i)", p=P
                            ),
                        )
                    for hh in range(2):
                        s0, s1 = hh * (NG // 2), (hh + 1) * (NG // 2) if hh == 0 else NG
                        ns = s1 - s0
                        et = exp_pool.tile([P, (NG - NG // 2) * P], bf16, tag="exp")
                        nc.vector.tensor_copy(
                            out=et[:, : ns * P].rearrange(
                                "p (s e) -> p s e", e=P
                            )[:, :, :D],
                            in_=dt_[:, s0 * D : s1 * D].rearrange(
                                "p (s i) -> p s i", i=D
                            ),
                        )
                        nc.scalar.dma_start(
                            out=table[c * SLOTS : (c + 1) * SLOTS, :]
                            .rearrange("(p s) e -> p s e", p=P)[:, s0:s1, :],
                            in_=et[:, : ns * P].rearrange(
                                "p (s e) -> p s e", e=P
                            ),
                        )

            if not USE_ST1:
                build_table(None)

            for k in range(1, ORDER + 1):
                wk = w[k]
                off_full = off_pool.tile([P, GF], f32, tag="off")
                for p2 in range(NPAIR):
                    nbp = int(nb_pair[p2])
                    bp = int(base_p[p2])

                    # Q for the whole pair, (t, g, b)-ordered
                    qg = q_pool.tile([P, max_nb_pair * DD], bf16, tag="qg")
                    if "qdma" not in ABLATE:
                        nc.sync.dma_start(
                            out=qg[:, : nbp * DD],
                            in_=qs2[:, bp * DD : (bp + nbp) * DD],
                        )

                    # one-hot scatter matrix for the whole pair
                    S8 = s_pool.tile([P, max_nb_pair * P], bf16, tag="S8")
                    if "s8" not in ABLATE:
                        nc.vector.tensor_tensor(
                            out=S8[:, : nbp * P].rearrange(
                                "p (b c) -> p b c", b=nbp
                            ),
                            in0=rowl_t[:, bp : bp + nbp]
                            .unsqueeze(2)
                            .to_broadcast([P, nbp, P]),
                            in1=iota_t[:].unsqueeze(1).to_broadcast([P, nbp, P]),
                            op=mybir.AluOpType.is_equal,
                        )

                    pacc = psum_pool.tile([P, 2 * DD], f32, tag="acc")

                    stg = {}
                    for t in range(N_CLS):
                        nb = int(nb_pt[p2, t])
                        if nb == 0:
                            continue
                        bpt = int(base_pt[p2, t])
                        if k == 1 and USE_ST1:
                            # step 1 gathers x = h: pre-gathered on host
                            st = stage_pool.tile(
                                [P, max_nb_pt * D], bf16, tag=f"s1{t}"
                            )
                            nc.sync.dma_start(
                                out=st[:, : nb * D],
                                in_=st1_in[:, bpt * D : (bpt + nb) * D],
                            )
                            in1 = (
                                st[:, : nb * D]
                                .rearrange("p (b j) -> p b j", j=D)
                                .unsqueeze(3)
                                .to_broadcast([P, nb, D, D])
                            )
                        else:
                            st = stage_pool.tile(
                                [P, max_nb_pt * P], bf16, tag=f"st{t}"
                            )
                            if "gather" not in ABLATE:
                                for b0 in range(0, nb, MAX_GBLK):
                                    bn = min(MAX_GBLK, nb - b0)
                                    nc.gpsimd.dma_gather(
                                        out_ap=st[
                                            :, b0 * P : (b0 + bn) * P
                                        ].rearrange("p (b e) -> p b e", e=P),
                                        in_ap=(
                                            table if k == 1 else ag_out[k - 2]
                                        )[t * SUBT : (t + 1) * SUBT, :],
                                        idxs_ap=idx_t[
                                            :,
                                            (bpt + b0) * 8 : (bpt + b0 + bn) * 8,
                                        ],
                                        num_idxs=bn * P,
                                        num_idxs_reg=get_nreg(bn * P),
                                        elem_size=P,
                                        queue_num=t % N_QUEUES,
                                    )
                            in1 = (
                                st[:, : nb * P]
                                .rearrange("p (b e) -> p b e", e=P)[:, :, :D]
                                .unsqueeze(3)
                                .to_broadcast([P, nb, D, D])
                            )

                        prod = prod_pool.tile(
                            [P, max_nb_pt * DD], bf16, tag=f"prod{t}"
                        )
                        if "prod" not in ABLATE:
                            nc.vector.tensor_tensor(
                                out=prod[:, : nb * DD].rearrange(
                                    "p (b j i) -> p b j i", b=nb, j=D
                                ),
                                in0=qg[
                                    :, (bpt - bp) * DD : (bpt - bp + nb) * DD
                                ].rearrange("p (b j i) -> p b j i", b=nb, j=D),
                                in1=in1,
                                op=mybir.AluOpType.mult,
                            )
                        stg[t] = (st, prod)

                    if "mm" not in ABLATE:
                        # per-g start/stop across the (t, g, b) block order
                        for gi, g in enumerate((2 * p2, 2 * p2 + 1)):
                            acc = pacc[:, gi * DD : (gi + 1) * DD]
                            g_blocks = []
                            for t in range(N_CLS):
                                nbg = int(NB_gt[g, t])
                                if nbg == 0 or t not in stg:
                                    continue
                                bgt = int(base_gt[g, t])
                                bpt = int(base_pt[p2, t])
                                for b in range(nbg):
                                    g_blocks.append((t, bgt - bpt + b, bgt + b))
                            for bi_, (t, loc_t, glob) in enumerate(g_blocks):
                                nc.tensor.matmul(
                                    out=acc,
                                    lhsT=S8[
                                        :, (glob - bp) * P : (glob - bp + 1) * P
                                    ],
                                    rhs=stg[t][1][
                                        :, loc_t * DD : (loc_t + 1) * DD
                                    ],
                                    start=(bi_ == 0),
                                    stop=(bi_ == len(g_blocks) - 1),
                                )
                        nc.vector.tensor_reduce(
                            out=off_full[
                                :, (2 * p2) * D : (2 * p2 + 2) * D
                            ].rearrange("p (u i) -> p u i", u=2),
                            in_=pacc[:].rearrange(
                                "p (u j i) -> p u i j", u=2, j=D
                            ),
                            axis=mybir.AxisListType.X,
                            op=mybir.AluOpType.add,
                        )

                # ---- node update ----
                tmp = upd_pool.tile([P, GF], f32, tag="tmp")
                nc.vector.tensor_tensor(
                    out=tmp[:].rearrange("p (s i) -> p s i", s=NG),
                    in0=x[:].rearrange("p (s i) -> p s i", s=NG),
                    in1=deg_t[:].unsqueeze(2).to_broadcast([P, NG, D]),
                    op=mybir.AluOpType.mult,
                )
                nc.vector.tensor_tensor(
                    out=tmp[:], in0=tmp[:], in1=off_full[:],
                    op=mybir.AluOpType.subtract,
                )
                nc.vector.tensor_scalar_mul(tmp[:], tmp[:], alpha)
                tmp2 = upd_pool.tile([P, GF], f32, tag="tmp2")
                if k == 1:
                    nc.vector.tensor_tensor(
                        out=tp[:], in0=tmp[:], in1=x[:], op=mybir.AluOpType.subtract
                    )
                    nc.vector.tensor_scalar_mul(r_t[:], x[:], w[0])
                    nc.vector.tensor_scalar_mul(tmp2[:], tp[:], wk)
                    nc.vector.tensor_tensor(
                        out=r_t[:], in0=r_t[:], in1=tmp2[:], op=mybir.AluOpType.add
                    )
                else:
                    nc.vector.tensor_tensor(
                        out=tmp[:], in0=tmp[:], in1=x[:], op=mybir.AluOpType.subtract
                    )
                    nc.vector.tensor_scalar_mul(tmp[:], tmp[:], 2.0)
                    nc.vector.tensor_tensor(
                        out=tp[:], in0=tmp[:], in1=tp[:], op=mybir.AluOpType.subtract
                    )
                    nc.vector.tensor_scalar_mul(tmp2[:], tp[:], wk)
                    nc.vector.tensor_tensor(
                        out=r_t[:], in0=r_t[:], in1=tmp2[:], op=mybir.AluOpType.add
                    )
                x, tp = tp, x

                if k < ORDER:
                    ct = cast_pool.tile([P, GF], bf16, tag="ct")
                    nc.vector.tensor_copy(out=ct[:], in_=x[:])
                    dst = ag_in[k - 1]
                    for hh in range(2):
                        s0 = hh * (NG // 2)
                        s1 = (hh + 1) * (NG // 2) if hh == 0 else NG
                        ns = s1 - s0
                        et = exp_pool.tile(
                            [P, (NG - NG // 2) * P], bf16, tag="exp"
                        )
                        nc.vector.tensor_copy(
                            out=et[:, : ns * P].rearrange(
                                "p (s e) -> p s e", e=P
                            )[:, :, :D],
                            in_=ct[:, s0 * D : s1 * D].rearrange(
                                "p (s i) -> p s i", i=D
                            ),
                        )
                        nc.scalar.dma_start(
                            out=dst[:]
                            .rearrange("(p s) e -> p s e", p=P)[:, s0:s1, :],
                            in_=et[:, : ns * P].rearrange(
                                "p (s e) -> p s e", e=P
                            ),
                        )
                    if "cc" not in ABLATE:
                        nc.gpsimd.collective_compute(
                            "AllGather",
                            mybir.AluOpType.bypass,
                            ins=[dst[:]],
                            outs=[ag_out[k - 1][:]],
                            replica_groups=[list(range(N_CORES))],
                        )

            nc.sync.dma_start(out=r_out[:], in_=r_t[:])

    nc.compile()
    return nc


# ---------------------------------------------------------------------------

_CACHE = {}


def get_nc(meta):
    key = ("nc", meta["nblk"], N_QUEUES, USE_ST1)
    if key not in _CACHE:
        nc = _build_nc(meta)
        _spill_excess_waits(nc)
        _CACHE[key] = nc
    return _CACHE[key]


def kernel(h, Q, coeffs, edge_index, lambda_max):
    import time as _time

    h = np.asarray(h)
    Q = np.asarray(Q)
    coeffs = np.asarray(coeffs)
    edge_index = np.asarray(edge_index)
    lambda_max = np.asarray(lambda_max)

    _t0 = _time.time()
    in_maps, meta = _preprocess(h, Q, coeffs, edge_index, lambda_max)
    _t1 = _time.time()
    nc = get_nc(meta)
    _t2 = _time.time()
    print(
        f"[kernel] preprocess {_t1-_t0:.1f}s  build+compile {_t2-_t1:.1f}s  "
        f"nblk={meta['nblk']}",
        flush=True,
    )

    res = run_bass_kernel_spmd(nc, in_maps, list(range(N_CORES)))
    print(f"[kernel] run {_time.time()-_t2:.1f}s", flush=True)

    out = np.empty((N, D), dtype=np.float32)
    for c in range(N_CORES):
        r = res.results[c]["r_out"]
        r3 = r.reshape(SLOTS, D)
        out[c * NPC : (c + 1) * NPC] = r3[:NPC]
    return out
